# revision 2
# baseline (speedup 1.0000x reference)
"""GraphUnetNoPool on 8 trn2 NeuronCores. v5: software-pipelined collectives.

Layer PE order (quadrants of mm1 x mm2):  A=mm1(m0,k0) C=mm1(m0,k1)
E=mm2(m0)+post -> fire AG_0   B=mm1(m1,k0) D=mm1(m1,k1) F=mm2(m1)+post ->
fire AG_1.  Consumption next layer: A' needs AG_0 (window ~T-fire(E)),
C' needs AG_1 (window ~dur(A')).  Node split m0:m1 (default 5:3) balances
the two windows at ~10-11us each, hiding the ~8us AllGather latency.

A's k-chunks are host-permuted group-major so fp8 DoubleRow pairs stay
adjacent across the asymmetric split.  Skip connections stay resident in
SBUF (no DRAM round-trip).  mm2 uses v-chunks as stationary so outputs land
in natural [row, feature] layout (no PE transposes).
"""

import numpy as np
from contextlib import ExitStack

import concourse.bass as bass
import concourse.tile as tile
from concourse import bacc, mybir
from concourse.bass_utils import run_bass_kernel_spmd

F32 = mybir.dt.float32
BF16 = mybir.dt.bfloat16
F8 = mybir.dt.float8e4

N, D, C, L = 8192, 256, 8, 7
S = N // C            # 1024 rows per core
KC = N // 128         # 64 k-chunks
MQ = S // 128         # 8 m-chunks per slab
GM = (4, 4)           # m-chunks per allgather group
U_SCALE = 32.0


def _pieces(c0, c1):
    """Split column range [c0*128, c1*128) into <=512-wide psum pieces."""
    out = []
    a = c0 * 128
    end = c1 * 128
    while a < end:
        w = min(512, end - a)
        out.append((a, a + w))
        a += w
    return out


def build_nc(n=N, d=D, c=C, n_layers=L, repeat=1, gm=GM, no_ag=False,
             ag_nodep=False):
    s = n // c
    kc = n // 128
    mq = s // 128
    dh_n = d // 128
    G = len(gm)
    assert sum(gm) == mq
    off_m = [sum(gm[:j]) for j in range(G)]          # m-chunk offsets
    gkc = [gm[j] * c for j in range(G)]              # k-chunks per group
    goff = [sum(gkc[:j]) for j in range(G)]          # global chunk offsets
    assert all(g % 2 == 0 for g in gkc)
    dr = mybir.MatmulPerfMode.DoubleRow
    relu = mybir.ActivationFunctionType.Relu

    nc = bacc.Bacc("TRN2", target_bir_lowering=False, debug=False, num_devices=c)

    a_dram = nc.dram_tensor("a_slab", [n, s], F32, kind="ExternalInput")
    u0_dram = nc.dram_tensor("u0", [n, d], F8, kind="ExternalInput")
    h0s_dram = nc.dram_tensor("h0_slab", [s, d], F32, kind="ExternalInput")
    dslab_dram = nc.dram_tensor("dinv_slab", [128, mq], F32, kind="ExternalInput")
    wt_dram = nc.dram_tensor("wt", [n_layers, d, d], BF16, kind="ExternalInput")
    bias_dram = nc.dram_tensor("bias_nat", [128, n_layers * d], F32,
                               kind="ExternalInput")
    out_dram = nc.dram_tensor("out", [4, s, d], F32, kind="ExternalOutput")

    with ExitStack() as ctx:
        tc = ctx.enter_context(tile.TileContext(nc))
        dram = ctx.enter_context(tc.tile_pool(name="dram", bufs=1, space="DRAM"))
        res = ctx.enter_context(tc.tile_pool(name="res", bufs=1))
        stage = ctx.enter_context(tc.tile_pool(name="stage", bufs=2))
        up = ctx.enter_context(tc.tile_pool(name="up", bufs=2))
        wtp = ctx.enter_context(tc.tile_pool(name="wtp", bufs=2))
        work = ctx.enter_context(tc.tile_pool(name="work", bufs=2))
        slabp = ctx.enter_context(tc.tile_pool(name="slabp", bufs=2))
        pmm1 = ctx.enter_context(tc.tile_pool(name="pmm1", bufs=1, space="PSUM"))
        post = ctx.enter_context(tc.tile_pool(name="post", bufs=2, space="PSUM"))

        # ---- persistent DRAM scratch ----
        ag_ins = [
            dram.tile([gm[j] * 128, d], F8, name=f"ag_in{j}", tag=f"ag_in{j}",
                      bufs=2)
            for j in range(G)
        ]
        ag_outs = [
            [
                dram.tile([gkc[j] * 128, d], F8, name=f"ag_out{i}_{j}",
                          tag=f"ag_out{i}_{j}", addr_space="Shared")
                for j in range(G)
            ]
            for i in range((n_layers - 1) * repeat)
        ]

        # ---- persistent SBUF ----
        a_sb = res.tile([128, kc, s], F8, name="a_sb")
        dinv32_sb = res.tile([128, mq], F32, name="dinv32_sb")
        dinvs_sb = res.tile([128, mq], F32, name="dinvs_sb")
        bias_bc = res.tile([128, n_layers, d], F32, name="bias_bc")
        skip_res = [
            res.tile([128, mq, d], F32, name=f"skip_res{i}") for i in range(3)
        ]

        dinv_tmp = res.tile([128, mq], F32, name="dinv_tmp")
        nc.sync.dma_start(out=dinv_tmp, in_=dslab_dram[:, :])
        nc.vector.tensor_scalar(
            out=dinv32_sb, in0=dinv_tmp, scalar1=U_SCALE, scalar2=None,
            op0=mybir.AluOpType.mult,
        )
        nc.vector.tensor_scalar(
            out=dinvs_sb, in0=dinv_tmp, scalar1=1.0 / U_SCALE, scalar2=None,
            op0=mybir.AluOpType.mult,
        )
        nc.sync.dma_start(
            out=bias_bc, in_=bias_dram[:, :].rearrange("p (l d2) -> p l d2", d2=d)
        )

        # ---- startup: load A column-slab (host pre-permuted), cast to fp8 ----
        for k in range(kc):
            st = stage.tile([128, s], F32, name="st", tag="stage")
            nc.sync.dma_start(out=st, in_=a_dram[k * 128 : (k + 1) * 128, :])
            nc.vector.tensor_copy(a_sb[:, k, :], st)

        skip_slot = {4: 2, 5: 1, 6: 0}
        u0_v = u0_dram[:, :].rearrange("(t p) d2 -> p t d2", p=128)

        for rep_l in range(n_layers * repeat):
            rep, l = divmod(rep_l, n_layers)
            # ---- stage U per gather group ----
            u_hi = [
                up.tile([128, gkc[j], d], F8, name=f"u_hi{j}", tag=f"u_hi{j}")
                for j in range(G)
            ]
            for j in range(G):
                if l == 0 or ag_nodep:
                    nc.sync.dma_start(
                        out=u_hi[j],
                        in_=u0_v[:, goff[j] : goff[j] + gkc[j], :],
                    )
                else:
                    src = ag_outs[rep * (n_layers - 1) + l - 1][j]
                    sv = src.rearrange("(t p) d2 -> p t d2", p=128)
                    nsp = 4 if j == 0 else 3
                    step = (gkc[j] + nsp - 1) // nsp
                    for i in range(nsp):
                        lo = i * step
                        hi = min(gkc[j], lo + step)
                        if lo >= hi:
                            break
                        nc.sync.dma_start(
                            out=u_hi[j][:, lo:hi, :], in_=sv[:, lo:hi, :]
                        )

            wt_t = wtp.tile([128, dh_n, d], BF16, name="wt_t", tag="wt")
            nc.scalar.dma_start(
                out=wt_t, in_=wt_dram[l].rearrange("(kc p) o -> p kc o", p=128)
            )

            # skip premultiply for NEXT layer's input (from resident skip)
            nl = l + 1
            skip_pre = None
            if nl in skip_slot and nl < n_layers:
                skip_pre = slabp.tile([128, mq, d], F32, name="skip_pre",
                                      tag="skip")
                for m in range(mq):
                    nc.vector.tensor_scalar(
                        out=skip_pre[:, m, :],
                        in0=skip_res[skip_slot[nl]][:, m, :],
                        scalar1=dinv32_sb[:, m : m + 1],
                        scalar2=None,
                        op0=mybir.AluOpType.mult,
                    )

            is_out = l >= n_layers - 3
            save_skip = l <= 2
            if save_skip:
                h_nat = skip_res[l]
            else:
                h_nat = slabp.tile([128, mq, d], F32, name="h_nat", tag="hnat",
                                   bufs=1)
            if l == n_layers - 1:
                h0s = slabp.tile([128, mq, d], F32, name="h0s", tag="skip")
                nc.scalar.dma_start(
                    out=h0s,
                    in_=h0s_dram[:, :].rearrange("(m p) d2 -> p m d2", p=128),
                )
                out3 = slabp.tile([128, mq, d], F32, name="out3", tag="us", bufs=1)
            else:
                us = slabp.tile([128, mq, d], F8, name="us", tag="us", bufs=1)

            v_sb = [
                work.tile([128, s], BF16, name="v_sb", tag="vsb")
                for _ in range(dh_n)
            ]

            # psum tiles per (m-group, piece); live through both k quadrants.
            # Full-bank [128, 512] allocations: matmul accumulation into
            # sub-bank psum tiles faults on hw; only [:, 0:width] is used.
            ph = [
                [
                    [pmm1.tile([128, 512], F32, name=f"ph{mg}",
                               tag=f"pmm1_{mg}_{pi}_{dh}")
                     for dh in range(dh_n)]
                    for pi, (p0, p1) in enumerate(
                        _pieces(off_m[mg], off_m[mg] + gm[mg]))
                ]
                for mg in range(G)
            ]

            def mm1_quadrant(mg, j):
                pieces = _pieces(off_m[mg], off_m[mg] + gm[mg])
                for tp_i in range(gkc[j] // 2):
                    t0 = 2 * tp_i
                    g0 = goff[j] + t0
                    st_ = (j == 0 and tp_i == 0)
                    sp_ = (j == G - 1 and tp_i == gkc[j] // 2 - 1)
                    for pi, (p0, p1) in enumerate(pieces):
                        rhs = a_sb[:, g0 : g0 + 2, p0:p1]
                        for dh in range(dh_n):
                            dsl = slice(dh * 128, (dh + 1) * 128)
                            nc.tensor.matmul(
                                ph[mg][pi][dh][:, 0 : p1 - p0],
                                u_hi[j][:, t0 : t0 + 2, dsl],
                                rhs,
                                start=st_, stop=sp_, perf_mode=dr,
                            )

            def mm2_post(mg):
                pieces = _pieces(off_m[mg], off_m[mg] + gm[mg])
                for pi, (p0, p1) in enumerate(pieces):
                    for dh in range(dh_n):
                        if dh == 0:
                            nc.scalar.copy(
                                v_sb[dh][:, p0:p1], ph[mg][pi][dh][:, 0 : p1 - p0]
                            )
                        else:
                            nc.vector.tensor_copy(
                                v_sb[dh][:, p0:p1], ph[mg][pi][dh][:, 0 : p1 - p0]
                            )
                for m in range(off_m[mg], off_m[mg] + gm[mg]):
                    psof = post.tile([128, 2 * d], F32, name="pso", tag="post")
                    pso = psof[:, 0:d]
                    for kin in range(dh_n):
                        nc.tensor.matmul(
                            pso,
                            v_sb[kin][:, m * 128 : (m + 1) * 128],
                            wt_t[:, kin, :],
                            start=(kin == 0),
                            stop=(kin == dh_n - 1),
                        )
                    nc.vector.scalar_tensor_tensor(
                        out=h_nat[:, m, :],
                        in0=pso,
                        scalar=dinvs_sb[:, m : m + 1],
                        in1=bias_bc[:, l, :],
                        op0=mybir.AluOpType.mult,
                        op1=mybir.AluOpType.add,
                    )
                    nc.scalar.activation(h_nat[:, m, :], h_nat[:, m, :], relu)
                    if l < n_layers - 1:
                        if skip_pre is not None:
                            nc.vector.scalar_tensor_tensor(
                                out=us[:, m, :],
                                in0=h_nat[:, m, :],
                                scalar=dinv32_sb[:, m : m + 1],
                                in1=skip_pre[:, m, :],
                                op0=mybir.AluOpType.mult,
                                op1=mybir.AluOpType.add,
                            )
                        else:
                            nc.vector.tensor_scalar(
                                out=us[:, m, :],
                                in0=h_nat[:, m, :],
                                scalar1=dinv32_sb[:, m : m + 1],
                                scalar2=None,
                                op0=mybir.AluOpType.mult,
                            )
                    else:
                        nc.vector.tensor_add(
                            out3[:, m, :], h_nat[:, m, :], h0s[:, m, :]
                        )
                if l < n_layers - 1:
                    agi = ag_ins[mg]
                    nc.gpsimd.dma_start(
                        out=agi.rearrange("(mm p) d2 -> p mm d2", p=128),
                        in_=us[:, off_m[mg] : off_m[mg] + gm[mg], :],
                    )
                    if not no_ag:
                        nc.gpsimd.collective_compute(
                            "AllGather",
                            mybir.AluOpType.bypass,
                            replica_groups=[list(range(c))],
                            ins=[agi.opt()],
                            outs=[ag_outs[rep * (n_layers - 1) + l][mg].opt()],
                        )
                    else:
                        nc.gpsimd.dma_start(
                            out=ag_outs[rep * (n_layers - 1) + l][mg][
                                0 : gm[mg] * 128, :
                            ],
                            in_=agi[:, :],
                        )

            # ---- the pipelined layer: A C E B D F ----
            mm1_quadrant(0, 0)
            mm1_quadrant(0, 1)
            mm2_post(0)
            mm1_quadrant(1, 0)
            mm1_quadrant(1, 1)
            mm2_post(1)

            # ---- DMAs out ----
            if is_out and l < n_layers - 1:
                nc.scalar.dma_start(
                    out=out_dram[l - (n_layers - 3)].rearrange(
                        "(m p) d2 -> p m d2", p=128
                    ),
                    in_=h_nat,
                )
            if l == n_layers - 1:
                nc.scalar.dma_start(
                    out=out_dram[2].rearrange("(m p) d2 -> p m d2", p=128),
                    in_=h_nat,
                )
                nc.scalar.dma_start(
                    out=out_dram[3].rearrange("(m p) d2 -> p m d2", p=128),
                    in_=out3,
                )

    nc.compile()
    return nc


try:
    import ml_dtypes

    ml_f8 = ml_dtypes.float8_e4m3fn
except ImportError:  # pragma: no cover
    import jax.numpy as jnp

    ml_f8 = jnp.float8_e4m3fn


def _perm(n, c, gm):
    """Global node permutation: group-major (group, core, chunk, row)."""
    s = n // c
    mq = s // 128
    off_m = [sum(gm[:j]) for j in range(len(gm))]
    order = []
    for j in range(len(gm)):
        for ci in range(c):
            for kk in range(gm[j]):
                m = off_m[j] + kk
                base = ci * s + m * 128
                order.append(np.arange(base, base + 128))
    return np.concatenate(order)


def prep_inputs(g, h, W_down, b_down, W_bottom, b_bottom, W_up, b_up, c=C,
                gm=GM):
    n = g.shape[0]
    s = n // c
    d = h.shape[1]
    g = np.asarray(g, np.float32)
    h = np.asarray(h, np.float32)
    deg = g.sum(axis=1) + 1.0
    dinv = (1.0 / np.sqrt(deg)).astype(np.float32)

    perm = _perm(n, c, gm)
    u0 = (h * dinv[:, None]).astype(np.float32)
    u0_packed = np.asarray((u0 * U_SCALE).astype(ml_f8))[perm]  # permuted rows

    Ws = [W_down[0], W_down[1], W_down[2], W_bottom, W_up[0], W_up[1], W_up[2]]
    bs = [b_down[0], b_down[1], b_down[2], b_bottom, b_up[0], b_up[1], b_up[2]]
    import ml_dtypes as _md

    wt = np.stack(
        [np.ascontiguousarray(np.asarray(W, np.float32).T) for W in Ws]
    ).astype(_md.bfloat16)
    nl = len(Ws)
    bias_nat = np.zeros((128, nl, d), np.float32)
    for li, b in enumerate(bs):
        bias_nat[:, li, :] = np.asarray(b, np.float32)[None, :]

    in_maps = []
    for ci in range(c):
        sl = slice(ci * s, (ci + 1) * s)
        a_slab = np.ascontiguousarray(g[:, sl])
        idx = np.arange(s)
        a_slab[ci * s + idx, idx] += 1.0  # fold self-loops into the slab
        a_slab = np.ascontiguousarray(a_slab[perm])  # group-major k rows
        dinv_slab = dinv[sl].reshape(s // 128, 128).T.copy()

        in_maps.append(
            dict(
                a_slab=a_slab,
                u0=u0_packed,
                h0_slab=np.ascontiguousarray(h[sl]),
                dinv_slab=dinv_slab,
                wt=wt,
                bias_nat=bias_nat.reshape(128, nl * d),
            )
        )
    return in_maps


_NC_CACHE = {}


def kernel(g, h, W_down, b_down, W_bottom, b_bottom, W_up, b_up):
    key = "full"
    if key not in _NC_CACHE:
        _NC_CACHE[key] = build_nc()
    nc = _NC_CACHE[key]
    in_maps = prep_inputs(g, h, W_down, b_down, W_bottom, b_bottom, W_up, b_up)
    res = run_bass_kernel_spmd(nc, in_maps, list(range(C)))
    outs = [np.asarray(r["out"]).reshape(4, S, D) for r in res.results]
    full = np.concatenate(outs, axis=1)  # [4, N, D]
    return full.astype(np.float32)


if __name__ == "__main__":
    import reference

    inputs = reference.setup_inputs()
    inputs = {k: np.asarray(v) for k, v in inputs.items()}
    out = kernel(**inputs)
    exp = np.asarray(reference.reference(**reference.setup_inputs()))
    err = np.abs(out - exp).max() / (np.abs(exp).max() + 1e-30)
    rel = np.linalg.norm(out - exp) / (np.linalg.norm(exp) + 1e-30)
    print("max-scaled err:", err, "rel l2:", rel)
